# revision 1
# baseline (speedup 1.0000x reference)
"""Trainium2 Bass kernel for nn_NativeAttention (dense transformer prefill).

Strategy (8 NeuronCores, tensor-parallel over heads):
  - Core c owns heads {2c, 2c+1}. Host pre-transposes weights/activations so
    every matmul operand is contraction-major (no on-chip transposes).
  - Per core: QKV projection (fp32r matmuls, full-rate), causal SDPA with
    scores computed TRANSPOSED (S^T = K @ Q^T, kv on partitions) so that
    softmax denominators come from a ones-matmul and the AV product
    y^T = V^T @ P needs no transposes, then the o_proj partial product.
  - Softmax is unnormalized (inputs are N(0,1)*0.02-scaled => |scores|<~6,
    exp never overflows fp32); rows are divided by the ones-matmul rowsum.
  - Each core returns its K^T / V (exact, fp32) and its o_proj PARTIAL
    (fp32). The host sums the 8 partials (cheap, exact) -- this replaces an
    expensive on-chip ReduceScatter.

This walrus build allows only ONE sync-wait per instruction; Tile emits
multi-wait instructions, so legalize_bir() hoists extra waits into
standalone EventSemaphore instructions before compiling.
"""

import json
import types

import numpy as np

import concourse.bass as bass
import concourse.mybir as mybir
import concourse.tile as tile
from concourse.bass_utils import run_bass_kernel_spmd

F32 = mybir.dt.float32
F32R = mybir.dt.float32r

B = 4
S = 2048
DM = 2048
H = 16
HD = 128
NCORES = 8
HPC = H // NCORES      # heads per core = 2
DHC = HPC * HD         # 256
KO = DM // 128         # 16 contraction blocks
XC = 256               # x seq-chunk width for projection
QC = 512               # q chunk width in SDPA
NQC = S // QC          # 4
SCALE = 1.0 / float(np.sqrt(HD))


def legalize_bir(bir_bytes: bytes) -> bytes:
    """Split multi-wait instructions (this walrus allows 1 wait/instruction)."""
    j = json.loads(bir_bytes)
    cnt = 0
    for f in j["functions"]:
        for bb in f["blocks"]:
            out = []
            for ins in bb["instructions"]:
                si = ins.get("sync_info")
                ow = (si or {}).get("on_wait") or []
                if len(ow) > 1:
                    for w in ow[:-1]:
                        cnt += 1
                        out.append({
                            "debug": ins.get("debug"),
                            "engine": ins["engine"],
                            "ins": [],
                            "name": f"LGZW-{cnt}",
                            "opcode": "EventSemaphore",
                            "outs": [],
                            "sync_info": {"on_update": [], "on_wait": [w]},
                        })
                    si["on_wait"] = [ow[-1]]
                out.append(ins)
            bb["instructions"] = out
    return json.dumps(j).encode()


def patch_legalize(nc: bass.Bass) -> None:
    orig = nc.to_json_bytes
    nc.to_json_bytes = types.MethodType(lambda self: legalize_bir(orig()), nc)


def build_nc() -> bass.Bass:
    nc = bass.Bass()
    xT = nc.dram_tensor("xT", [B, DM, S], F32, kind="ExternalInput")
    wqkvT = nc.dram_tensor("wqkvT", [DM, 3 * DHC], F32, kind="ExternalInput")
    woT = nc.dram_tensor("woT", [DHC, DM], F32, kind="ExternalInput")
    masks = nc.dram_tensor("masks", [4, 128, QC], F32, kind="ExternalInput")
    ktout = nc.dram_tensor("ktout", [B, DHC, S], F32, kind="ExternalOutput")
    vout = nc.dram_tensor("vout", [B, S, DHC], F32, kind="ExternalOutput")
    ypart = nc.dram_tensor("ypart", [B, S, DM], F32, kind="ExternalOutput")

    with tile.TileContext(nc) as tc:
        with (
            tc.tile_pool(name="const", bufs=1) as constp,
            tc.tile_pool(name="xcp", bufs=2) as xcp,
            tc.tile_pool(name="qkv", bufs=1) as qkvp,
            tc.tile_pool(name="work", bufs=3) as workp,
            tc.tile_pool(name="small", bufs=2) as smallp,
            tc.tile_pool(name="yop", bufs=3) as yop,
            tc.tile_pool(name="ps_mm", bufs=4, space="PSUM") as ps_mm,
            tc.tile_pool(name="ps_acc", bufs=2, space="PSUM") as ps_acc,
            tc.tile_pool(name="ps_l", bufs=2, space="PSUM") as ps_l,
        ):
            # ---- constants ----
            wq_sb = constp.tile([128, KO, 3 * DHC], F32R, tag="wqkv")
            nc.sync.dma_start(
                wq_sb[:],
                wqkvT[:, :].rearrange("(ko p) m -> p ko m", p=128).bitcast(F32R),
            )
            wo_sb = constp.tile([128, HPC, DM], F32R, tag="wo")
            nc.sync.dma_start(
                wo_sb[:],
                woT[:, :].rearrange("(k p) m -> p k m", p=128).bitcast(F32R),
            )
            mask_sb = constp.tile([128, 4, QC], F32R, tag="mask")
            nc.sync.dma_start(
                mask_sb[:],
                masks[:, :, :].rearrange("i p f -> p i f").bitcast(F32R),
            )
            ones_sb = constp.tile([128, 1], F32, tag="ones")
            nc.vector.memset(ones_sb[:], 1.0)
            ones1_sb = constp.tile([1, 128], F32, tag="ones1")
            nc.vector.memset(ones1_sb[:], 1.0)

            for b in range(B):
                # per-batch activation tiles (tags reuse slots across batches)
                qt = qkvp.tile([128, HPC, S], F32R, tag="qt")
                kt = qkvp.tile([128, HPC, S], F32R, tag="kt")
                vv = qkvp.tile([128, S // 128, DHC], F32R, tag="vv")
                ytn = qkvp.tile([128, HPC, S], F32R, tag="ytn")

                # ---- stage A: QKV projection ----
                for sc in range(S // XC):
                    xc = xcp.tile([128, KO, XC], F32R, tag="xc")
                    nc.sync.dma_start(
                        xc[:],
                        xT[b, :, sc * XC:(sc + 1) * XC]
                        .rearrange("(ko p) s -> p ko s", p=128)
                        .bitcast(F32R),
                    )
                    # Q^T and K^T: [dh-block 128, seq-chunk]
                    for m in range(2 * HPC):
                        pq = ps_mm.tile([128, QC], F32, tag="mm")
                        for ko in range(KO):
                            nc.tensor.matmul(
                                pq[:, :XC],
                                wq_sb[:, ko, m * 128:(m + 1) * 128],
                                xc[:, ko, :],
                                start=(ko == 0),
                                stop=(ko == KO - 1),
                            )
                        dst = qt if m < HPC else kt
                        h = m % HPC
                        with nc.allow_low_precision(reason="f32r activations"):
                            nc.scalar.copy(
                                dst[:, h, sc * XC:(sc + 1) * XC], pq[:, :XC]
                            )
                    # V natural: [seq-block 128, dh 256]
                    for sm in range(XC // 128):
                        pv = ps_mm.tile([128, QC], F32, tag="mm")
                        for ko in range(KO):
                            nc.tensor.matmul(
                                pv[:, :DHC],
                                xc[:, ko, sm * 128:(sm + 1) * 128],
                                wq_sb[:, ko, 2 * DHC:3 * DHC],
                                start=(ko == 0),
                                stop=(ko == KO - 1),
                            )
                        blk = sc * (XC // 128) + sm
                        with nc.allow_low_precision(reason="f32r activations"):
                            nc.scalar.copy(vv[:, blk, :], pv[:, :DHC])

                # K^T / V outputs (exact fp32 bits)
                nc.sync.dma_start(
                    ktout[b].rearrange("(h p) s -> p h s", p=128).bitcast(F32R),
                    kt[:],
                )
                nc.sync.dma_start(
                    vout[b].rearrange("(n p) d -> p n d", p=128).bitcast(F32R),
                    vv[:],
                )

                # ---- stage B: causal SDPA (S^T orientation) ----
                for h in range(HPC):
                    for qc in range(NQC):
                        nblk = 4 * qc + 4
                        py = ps_acc.tile([128, QC], F32, tag="acc")
                        tt = smallp.tile([128, QC], F32R, tag="t")
                        for j in range(nblk):
                            pss = ps_mm.tile([128, QC], F32, tag="mm")
                            nc.tensor.matmul(
                                pss[:],
                                kt[:, h, j * 128:(j + 1) * 128],
                                qt[:, h, qc * QC:(qc + 1) * QC],
                                start=True,
                                stop=True,
                            )
                            p_sb = workp.tile([128, QC], F32R, tag="p")
                            with nc.allow_low_precision(reason="f32r probs"):
                                nc.scalar.activation(
                                    p_sb[:],
                                    pss[:],
                                    mybir.ActivationFunctionType.Exp,
                                    scale=SCALE,
                                )
                                if j >= 4 * qc:  # diagonal block: causal mask
                                    nc.vector.tensor_mul(
                                        p_sb[:], p_sb[:], mask_sb[:, j - 4 * qc, :]
                                    )
                                if j == 0:
                                    nc.vector.tensor_copy(tt[:], p_sb[:])
                                else:
                                    nc.vector.tensor_add(tt[:], tt[:], p_sb[:])
                            nc.tensor.matmul(
                                py[:],
                                vv[:, j, h * 128:(h + 1) * 128],
                                p_sb[:],
                                start=(j == 0),
                                stop=(j == nblk - 1),
                            )
                        # denominators: ones-matmul over partitions, recip,
                        # broadcast back to 128 partitions via K=1 matmul
                        pl = ps_l.tile([1, QC], F32, tag="l")
                        nc.tensor.matmul(
                            pl[:], ones_sb[:].bitcast(F32R), tt[:],
                            start=True, stop=True,
                        )
                        l_sb = smallp.tile([1, QC], F32R, tag="lr")
                        with nc.allow_low_precision(reason="f32r recip"):
                            nc.vector.reciprocal(l_sb[:], pl[:])
                        pbc = ps_mm.tile([128, QC], F32, tag="mm")
                        nc.tensor.matmul(
                            pbc[:], ones1_sb[:].bitcast(F32R), l_sb[:],
                            start=True, stop=True,
                        )
                        lbc = smallp.tile([128, QC], F32, tag="lbc")
                        nc.scalar.copy(lbc[:], pbc[:])
                        with nc.allow_low_precision(reason="f32r yT"):
                            nc.vector.tensor_mul(
                                ytn[:, h, qc * QC:(qc + 1) * QC], py[:], lbc[:]
                            )

                # ---- stage C: o_proj partial ----
                for sm in range(S // 128):
                    for n in range(DM // QC):
                        po = ps_mm.tile([128, QC], F32, tag="mm")
                        for k2 in range(HPC):
                            nc.tensor.matmul(
                                po[:],
                                ytn[:, k2, sm * 128:(sm + 1) * 128],
                                wo_sb[:, k2, n * QC:(n + 1) * QC],
                                start=(k2 == 0),
                                stop=(k2 == HPC - 1),
                            )
                        yo = yop.tile([128, QC], F32, tag="yo")
                        nc.vector.tensor_copy(yo[:], po[:])
                        nc.sync.dma_start(
                            ypart[b, sm * 128:(sm + 1) * 128,
                                  n * QC:(n + 1) * QC],
                            yo[:],
                        )

    patch_legalize(nc)
    return nc


def _build_masks() -> np.ndarray:
    m = np.zeros((4, 128, QC), np.float32)
    f = np.arange(QC)[None, :]
    p = np.arange(128)[:, None]
    for i in range(4):
        m[i] = (f >= p + i * 128).astype(np.float32)
    return m


_NC_CACHE = None


def kernel(x, Wq, Wk, Wv, Wo):
    global _NC_CACHE
    x = np.ascontiguousarray(np.asarray(x, np.float32))
    Wq = np.ascontiguousarray(np.asarray(Wq, np.float32))
    Wk = np.ascontiguousarray(np.asarray(Wk, np.float32))
    Wv = np.ascontiguousarray(np.asarray(Wv, np.float32))
    Wo = np.ascontiguousarray(np.asarray(Wo, np.float32))

    xT = np.ascontiguousarray(x.transpose(0, 2, 1))
    masks = _build_masks()

    in_maps = []
    for c in range(NCORES):
        sl = slice(c * DHC, (c + 1) * DHC)
        wqkvT = np.ascontiguousarray(
            np.concatenate([Wq[sl].T, Wk[sl].T, Wv[sl].T], axis=1)
        )
        woT = np.ascontiguousarray(Wo[:, sl].T)
        in_maps.append(
            {"xT": xT, "wqkvT": wqkvT, "woT": woT, "masks": masks}
        )

    if _NC_CACHE is None:
        _NC_CACHE = build_nc()
    res = run_bass_kernel_spmd(
        _NC_CACHE, in_maps, core_ids=list(range(NCORES))
    ).results

    # assemble full outputs
    y = np.zeros((B, S, DM), np.float32)
    for c in range(NCORES):
        y += res[c]["ypart"]

    k = np.zeros((B, H, S, HD), np.float32)
    v = np.zeros((B, H, S, HD), np.float32)
    for c in range(NCORES):
        kt = res[c]["ktout"]  # [B, DHC, S]
        vc = res[c]["vout"]   # [B, S, DHC]
        for j in range(HPC):
            h = c * HPC + j
            k[:, h] = kt[:, j * HD:(j + 1) * HD, :].transpose(0, 2, 1)
            v[:, h] = vc[:, :, j * HD:(j + 1) * HD]
    return (y, k, v)


# revision 3
# speedup vs baseline: 145.3288x; 145.3288x over previous
"""Trainium2 Bass kernel for nn_NativeAttention (dense transformer prefill).

Strategy (8 NeuronCores, tensor-parallel over heads):
  - Core c owns heads {2c, 2c+1}. Host pre-transposes weights/activations so
    every matmul operand is contraction-major (no on-chip transposes).
  - Per core: QKV projection (fp32r matmuls, full-rate), causal SDPA with
    scores computed TRANSPOSED (S^T = K @ Q^T, kv on partitions) so that
    softmax denominators come from a ones-matmul and the AV product
    y^T = V^T @ P needs no transposes, then the o_proj partial product.
  - Softmax is unnormalized (inputs are N(0,1)*0.02-scaled => |scores|<~6,
    exp never overflows fp32); rows are divided by the ones-matmul rowsum.
  - Each core returns its K^T / V (exact, fp32) and its o_proj PARTIAL
    (fp32). The host sums the 8 partials (cheap, exact) -- this replaces an
    expensive on-chip ReduceScatter.

This walrus build allows only ONE sync-wait per instruction; Tile emits
multi-wait instructions, so legalize_bir() hoists extra waits into
standalone EventSemaphore instructions before compiling.
"""

import json
import types

import numpy as np

import concourse.bass as bass
import concourse.mybir as mybir
import concourse.tile as tile
from concourse.bass_utils import run_bass_kernel_spmd

F32 = mybir.dt.float32
F32R = mybir.dt.float32r

B = 4
S = 2048
DM = 2048
H = 16
HD = 128
NCORES = 8
HPC = H // NCORES      # heads per core = 2
DHC = HPC * HD         # 256
KO = DM // 128         # 16 contraction blocks
XC = 256               # x seq-chunk width for projection
QC = 512               # q chunk width in SDPA
NQC = S // QC          # 4
SCALE = 1.0 / float(np.sqrt(HD))


def legalize_bir(bir_bytes: bytes) -> bytes:
    """Split multi-wait instructions (this walrus allows 1 wait/instruction)."""
    j = json.loads(bir_bytes)
    cnt = 0
    for f in j["functions"]:
        for bb in f["blocks"]:
            out = []
            for ins in bb["instructions"]:
                si = ins.get("sync_info")
                ow = (si or {}).get("on_wait") or []
                if len(ow) > 1:
                    for w in ow[:-1]:
                        cnt += 1
                        out.append({
                            "debug": ins.get("debug"),
                            "engine": ins["engine"],
                            "ins": [],
                            "name": f"LGZW-{cnt}",
                            "opcode": "EventSemaphore",
                            "outs": [],
                            "sync_info": {"on_update": [], "on_wait": [w]},
                        })
                    si["on_wait"] = [ow[-1]]
                out.append(ins)
            bb["instructions"] = out
    return json.dumps(j).encode()


def patch_legalize(nc: bass.Bass) -> None:
    orig = nc.to_json_bytes
    nc.to_json_bytes = types.MethodType(lambda self: legalize_bir(orig()), nc)


def build_nc(repeat: int = 1) -> bass.Bass:
    """repeat>1 builds a NEFF that executes the whole computation `repeat`
    times back-to-back (same buffers) — used by test.py to measure HW time
    differentially through the axon tunnel's large dispatch overhead."""
    nc = bass.Bass()
    xT = nc.dram_tensor("xT", [B, DM, S], F32, kind="ExternalInput")
    wqkvT = nc.dram_tensor("wqkvT", [DM, 3 * DHC], F32, kind="ExternalInput")
    woT = nc.dram_tensor("woT", [DHC, DM], F32, kind="ExternalInput")
    masks = nc.dram_tensor("masks", [4, 128, QC], F32, kind="ExternalInput")
    ktout = nc.dram_tensor("ktout", [B, DHC, S], F32, kind="ExternalOutput")
    vout = nc.dram_tensor("vout", [B, S, DHC], F32, kind="ExternalOutput")
    ypart = nc.dram_tensor("ypart", [B, S, DM], F32, kind="ExternalOutput")

    with tile.TileContext(nc) as tc:
        with (
            tc.tile_pool(name="const", bufs=1) as constp,
            tc.tile_pool(name="xcp", bufs=2) as xcp,
            tc.tile_pool(name="qkv", bufs=1) as qkvp,
            tc.tile_pool(name="work", bufs=3) as workp,
            tc.tile_pool(name="small", bufs=2) as smallp,
            tc.tile_pool(name="yop", bufs=3) as yop,
            tc.tile_pool(name="ps_mm", bufs=4, space="PSUM") as ps_mm,
            tc.tile_pool(name="ps_acc", bufs=2, space="PSUM") as ps_acc,
            tc.tile_pool(name="ps_l", bufs=2, space="PSUM") as ps_l,
        ):
            # ---- constants ----
            wq_sb = constp.tile([128, KO, 3 * DHC], F32R, tag="wqkv")
            nc.sync.dma_start(
                wq_sb[:],
                wqkvT[:, :].rearrange("(ko p) m -> p ko m", p=128).bitcast(F32R),
            )
            wo_sb = constp.tile([128, HPC, DM], F32R, tag="wo")
            nc.sync.dma_start(
                wo_sb[:],
                woT[:, :].rearrange("(k p) m -> p k m", p=128).bitcast(F32R),
            )
            mask_sb = constp.tile([128, 4, QC], F32R, tag="mask")
            nc.sync.dma_start(
                mask_sb[:],
                masks[:, :, :].rearrange("i p f -> p i f").bitcast(F32R),
            )
            ones_sb = constp.tile([128, 1], F32, tag="ones")
            nc.vector.memset(ones_sb[:], 1.0)
            ones1_sb = constp.tile([1, 128], F32, tag="ones1")
            nc.vector.memset(ones1_sb[:], 1.0)

            for b in [bb for _ in range(repeat) for bb in range(B)]:
                # per-batch activation tiles (tags reuse slots across batches)
                qt = qkvp.tile([128, HPC, S], F32R, tag="qt")
                kt = qkvp.tile([128, HPC, S], F32R, tag="kt")
                vv = qkvp.tile([128, S // 128, DHC], F32R, tag="vv")
                ytn = qkvp.tile([128, HPC, S], F32R, tag="ytn")

                # ---- stage A: QKV projection ----
                for sc in range(S // XC):
                    xc = xcp.tile([128, KO, XC], F32R, tag="xc")
                    nc.sync.dma_start(
                        xc[:],
                        xT[b, :, sc * XC:(sc + 1) * XC]
                        .rearrange("(ko p) s -> p ko s", p=128)
                        .bitcast(F32R),
                    )
                    # Q^T and K^T: [dh-block 128, seq-chunk]
                    for m in range(2 * HPC):
                        pq = ps_mm.tile([128, QC], F32, tag="mm")
                        for ko in range(KO):
                            nc.tensor.matmul(
                                pq[:, :XC],
                                wq_sb[:, ko, m * 128:(m + 1) * 128],
                                xc[:, ko, :],
                                start=(ko == 0),
                                stop=(ko == KO - 1),
                            )
                        dst = qt if m < HPC else kt
                        h = m % HPC
                        with nc.allow_low_precision(reason="f32r activations"):
                            nc.scalar.copy(
                                dst[:, h, sc * XC:(sc + 1) * XC], pq[:, :XC]
                            )
                    # V natural: [seq-block 128, dh 256]
                    for sm in range(XC // 128):
                        pv = ps_mm.tile([128, QC], F32, tag="mm")
                        for ko in range(KO):
                            nc.tensor.matmul(
                                pv[:, :DHC],
                                xc[:, ko, sm * 128:(sm + 1) * 128],
                                wq_sb[:, ko, 2 * DHC:3 * DHC],
                                start=(ko == 0),
                                stop=(ko == KO - 1),
                            )
                        blk = sc * (XC // 128) + sm
                        with nc.allow_low_precision(reason="f32r activations"):
                            nc.scalar.copy(vv[:, blk, :], pv[:, :DHC])

                # K^T / V outputs (exact fp32 bits)
                nc.sync.dma_start(
                    ktout[b].rearrange("(h p) s -> p h s", p=128).bitcast(F32R),
                    kt[:],
                )
                nc.sync.dma_start(
                    vout[b].rearrange("(n p) d -> p n d", p=128).bitcast(F32R),
                    vv[:],
                )

                # ---- stage B: causal SDPA (S^T orientation) ----
                for h in range(HPC):
                    for qc in range(NQC):
                        nblk = 4 * qc + 4
                        py = ps_acc.tile([128, QC], F32, tag="acc")
                        tt = smallp.tile([128, QC], F32R, tag="t")
                        for j in range(nblk):
                            pss = ps_mm.tile([128, QC], F32, tag="mm")
                            nc.tensor.matmul(
                                pss[:],
                                kt[:, h, j * 128:(j + 1) * 128],
                                qt[:, h, qc * QC:(qc + 1) * QC],
                                start=True,
                                stop=True,
                            )
                            p_sb = workp.tile([128, QC], F32R, tag="p")
                            with nc.allow_low_precision(reason="f32r probs"):
                                nc.scalar.activation(
                                    p_sb[:],
                                    pss[:],
                                    mybir.ActivationFunctionType.Exp,
                                    scale=SCALE,
                                )
                                if j >= 4 * qc:  # diagonal block: causal mask
                                    nc.vector.tensor_mul(
                                        p_sb[:], p_sb[:], mask_sb[:, j - 4 * qc, :]
                                    )
                                if j == 0:
                                    nc.vector.tensor_copy(tt[:], p_sb[:])
                                else:
                                    nc.vector.tensor_add(tt[:], tt[:], p_sb[:])
                            nc.tensor.matmul(
                                py[:],
                                vv[:, j, h * 128:(h + 1) * 128],
                                p_sb[:],
                                start=(j == 0),
                                stop=(j == nblk - 1),
                            )
                        # denominators: ones-matmul over partitions, recip,
                        # broadcast back to 128 partitions via K=1 matmul
                        pl = ps_l.tile([1, QC], F32, tag="l")
                        nc.tensor.matmul(
                            pl[:], ones_sb[:].bitcast(F32R), tt[:],
                            start=True, stop=True,
                        )
                        l_sb = smallp.tile([1, QC], F32R, tag="lr")
                        with nc.allow_low_precision(reason="f32r recip"):
                            nc.vector.reciprocal(l_sb[:], pl[:])
                        pbc = ps_mm.tile([128, QC], F32, tag="mm")
                        nc.tensor.matmul(
                            pbc[:], ones1_sb[:].bitcast(F32R), l_sb[:],
                            start=True, stop=True,
                        )
                        lbc = smallp.tile([128, QC], F32, tag="lbc")
                        nc.scalar.copy(lbc[:], pbc[:])
                        with nc.allow_low_precision(reason="f32r yT"):
                            nc.vector.tensor_mul(
                                ytn[:, h, qc * QC:(qc + 1) * QC], py[:], lbc[:]
                            )

                # ---- stage C: o_proj partial ----
                for sm in range(S // 128):
                    for n in range(DM // QC):
                        po = ps_mm.tile([128, QC], F32, tag="mm")
                        for k2 in range(HPC):
                            nc.tensor.matmul(
                                po[:],
                                ytn[:, k2, sm * 128:(sm + 1) * 128],
                                wo_sb[:, k2, n * QC:(n + 1) * QC],
                                start=(k2 == 0),
                                stop=(k2 == HPC - 1),
                            )
                        yo = yop.tile([128, QC], F32, tag="yo")
                        nc.vector.tensor_copy(yo[:], po[:])
                        nc.sync.dma_start(
                            ypart[b, sm * 128:(sm + 1) * 128,
                                  n * QC:(n + 1) * QC],
                            yo[:],
                        )

    patch_legalize(nc)
    return nc


def _build_masks() -> np.ndarray:
    m = np.zeros((4, 128, QC), np.float32)
    f = np.arange(QC)[None, :]
    p = np.arange(128)[:, None]
    for i in range(4):
        m[i] = (f >= p + i * 128).astype(np.float32)
    return m


_NC_CACHE = None


def kernel(x, Wq, Wk, Wv, Wo):
    global _NC_CACHE
    x = np.ascontiguousarray(np.asarray(x, np.float32))
    Wq = np.ascontiguousarray(np.asarray(Wq, np.float32))
    Wk = np.ascontiguousarray(np.asarray(Wk, np.float32))
    Wv = np.ascontiguousarray(np.asarray(Wv, np.float32))
    Wo = np.ascontiguousarray(np.asarray(Wo, np.float32))

    xT = np.ascontiguousarray(x.transpose(0, 2, 1))
    masks = _build_masks()

    in_maps = []
    for c in range(NCORES):
        sl = slice(c * DHC, (c + 1) * DHC)
        wqkvT = np.ascontiguousarray(
            np.concatenate([Wq[sl].T, Wk[sl].T, Wv[sl].T], axis=1)
        )
        woT = np.ascontiguousarray(Wo[:, sl].T)
        in_maps.append(
            {"xT": xT, "wqkvT": wqkvT, "woT": woT, "masks": masks}
        )

    if _NC_CACHE is None:
        _NC_CACHE = build_nc()
    res = run_bass_kernel_spmd(
        _NC_CACHE, in_maps, core_ids=list(range(NCORES))
    ).results

    # assemble full outputs
    y = np.zeros((B, S, DM), np.float32)
    for c in range(NCORES):
        y += res[c]["ypart"]

    k = np.zeros((B, H, S, HD), np.float32)
    v = np.zeros((B, H, S, HD), np.float32)
    for c in range(NCORES):
        kt = res[c]["ktout"]  # [B, DHC, S]
        vc = res[c]["vout"]   # [B, S, DHC]
        for j in range(HPC):
            h = c * HPC + j
            k[:, h] = kt[:, j * HD:(j + 1) * HD, :].transpose(0, 2, 1)
            v[:, h] = vc[:, :, j * HD:(j + 1) * HD]
    return (y, k, v)


# revision 8
# speedup vs baseline: 162.1910x; 1.1160x over previous
"""Trainium2 Bass kernel for nn_NativeAttention (dense transformer prefill).

Strategy (8 NeuronCores, tensor-parallel over heads):
  - Core c owns heads {2c, 2c+1}. Host pre-transposes weights/activations so
    every matmul operand is contraction-major (no on-chip transposes).
  - Per core: QKV projection (fp32r matmuls, full-rate), causal SDPA with
    scores computed TRANSPOSED (S^T = K @ Q^T, kv on partitions) so that
    softmax denominators come from a ones-matmul and the AV product
    y^T = V^T @ P needs no transposes, then the o_proj partial product.
  - Softmax is unnormalized (inputs are N(0,1)*0.02-scaled => |scores|<~6,
    exp never overflows fp32); rows are divided by the ones-matmul rowsum.
  - Each core returns its K^T / V (exact, fp32) and its o_proj PARTIAL
    (fp32). The host sums the 8 partials (cheap, exact) -- this replaces an
    expensive on-chip ReduceScatter.

This walrus build allows only ONE sync-wait per instruction; Tile emits
multi-wait instructions, so legalize_bir() hoists extra waits into
standalone EventSemaphore instructions before compiling.
"""

import json
import types

import numpy as np

import concourse.bass as bass
import concourse.mybir as mybir
import concourse.tile as tile
from concourse.bass_utils import run_bass_kernel_spmd

F32 = mybir.dt.float32
F32R = mybir.dt.float32r

B = 4
S = 2048
DM = 2048
H = 16
HD = 128
NCORES = 8
HPC = H // NCORES      # heads per core = 2
DHC = HPC * HD         # 256
KO = DM // 128         # 16 contraction blocks
XC = 256               # x seq-chunk width for projection
QC = 512               # q chunk width in SDPA
NQC = S // QC          # 4
SCALE = 1.0 / float(np.sqrt(HD))


def legalize_bir(bir_bytes: bytes) -> bytes:
    """Split multi-wait instructions (this walrus allows 1 wait/instruction)."""
    j = json.loads(bir_bytes)
    cnt = 0
    for f in j["functions"]:
        for bb in f["blocks"]:
            out = []
            for ins in bb["instructions"]:
                si = ins.get("sync_info")
                ow = (si or {}).get("on_wait") or []
                if len(ow) > 1:
                    for w in ow[:-1]:
                        cnt += 1
                        out.append({
                            "debug": ins.get("debug"),
                            "engine": ins["engine"],
                            "ins": [],
                            "name": f"LGZW-{cnt}",
                            "opcode": "EventSemaphore",
                            "outs": [],
                            "sync_info": {"on_update": [], "on_wait": [w]},
                        })
                    si["on_wait"] = [ow[-1]]
                out.append(ins)
            bb["instructions"] = out
    return json.dumps(j).encode()


def patch_legalize(nc: bass.Bass) -> None:
    orig = nc.to_json_bytes
    nc.to_json_bytes = types.MethodType(lambda self: legalize_bir(orig()), nc)


def build_nc(repeat: int = 1, compute_only: bool = False) -> bass.Bass:
    """repeat>1 builds a NEFF that executes the whole computation `repeat`
    times back-to-back (same buffers) — used by test.py to measure HW time
    differentially through the axon tunnel's large dispatch overhead.
    compute_only=True replaces all bulk DMA with static zero tiles
    (timing diagnostic: isolates compute from DMA)."""
    nc = bass.Bass()
    xT = nc.dram_tensor("xT", [B, DM, S], F32, kind="ExternalInput")
    wqkvT = nc.dram_tensor("wqkvT", [DM, 3 * DHC], F32, kind="ExternalInput")
    woT = nc.dram_tensor("woT", [DHC, DM], F32, kind="ExternalInput")
    masks = nc.dram_tensor("masks", [4, 128, QC], F32, kind="ExternalInput")
    ktout = nc.dram_tensor("ktout", [B, DHC, S], F32, kind="ExternalOutput")
    vout = nc.dram_tensor("vout", [B, S, DHC], F32, kind="ExternalOutput")
    ypart = nc.dram_tensor("ypart", [B, S, DM], F32, kind="ExternalOutput")

    with tile.TileContext(nc) as tc:
        with (
            tc.tile_pool(name="const", bufs=1) as constp,
            tc.tile_pool(name="xcp", bufs=2) as xcp,
            tc.tile_pool(name="qkv", bufs=1) as qkvp,
            tc.tile_pool(name="work", bufs=3) as workp,
            tc.tile_pool(name="small", bufs=2) as smallp,
            tc.tile_pool(name="yop", bufs=3) as yop,
            tc.tile_pool(name="ps_mm", bufs=4, space="PSUM") as ps_mm,
            tc.tile_pool(name="ps_acc", bufs=2, space="PSUM") as ps_acc,
            tc.tile_pool(name="ps_l", bufs=2, space="PSUM") as ps_l,
        ):
            # ---- constants ----
            wq_sb = constp.tile([128, KO, 3 * DHC], F32R, tag="wqkv")
            nc.sync.dma_start(
                wq_sb[:],
                wqkvT[:, :].rearrange("(ko p) m -> p ko m", p=128).bitcast(F32R),
            )
            wo_sb = constp.tile([128, HPC, DM], F32R, tag="wo")
            nc.sync.dma_start(
                wo_sb[:],
                woT[:, :].rearrange("(k p) m -> p k m", p=128).bitcast(F32R),
            )
            mask_sb = constp.tile([128, 4, QC], F32R, tag="mask")
            nc.sync.dma_start(
                mask_sb[:],
                masks[:, :, :].rearrange("i p f -> p i f").bitcast(F32R),
            )
            ones_sb = constp.tile([128, 1], F32, tag="ones")
            nc.vector.memset(ones_sb[:], 1.0)
            ones1_sb = constp.tile([1, 128], F32, tag="ones1")
            nc.vector.memset(ones1_sb[:], 1.0)

            for b in [bb for _ in range(repeat) for bb in range(B)]:
                # per-batch activation tiles (tags reuse slots across batches)
                qt = qkvp.tile([128, HPC, S], F32R, tag="qt")
                kt = qkvp.tile([128, HPC, S], F32R, tag="kt")
                vv = qkvp.tile([128, S // 128, DHC], F32R, tag="vv")
                ytn = qkvp.tile([128, HPC, S], F32R, tag="ytn")

                # ---- stage A: QKV projection ----
                for sc in range(S // XC):
                    xc = xcp.tile([128, KO, XC], F32R, tag="xc")
                    if compute_only:
                        # every allocation needs a writer for Tile; seed the
                        # two slots with zeros once, then 1-elem touches
                        if b == 0 and sc < 2:
                            nc.vector.memset(xc[:].bitcast(F32), 0.0)
                        else:
                            nc.vector.memset(xc[:, :1, :1].bitcast(F32), 0.0)
                    else:
                        nc.sync.dma_start(
                            xc[:],
                            xT[b, :, sc * XC:(sc + 1) * XC]
                            .rearrange("(ko p) s -> p ko s", p=128)
                            .bitcast(F32R),
                        )
                    # Q^T and K^T: [dh-block 128, seq-chunk]
                    for m in range(2 * HPC):
                        pq = ps_mm.tile([128, QC], F32, tag="mm")
                        for ko in range(KO):
                            nc.tensor.matmul(
                                pq[:, :XC],
                                wq_sb[:, ko, m * 128:(m + 1) * 128],
                                xc[:, ko, :],
                                start=(ko == 0),
                                stop=(ko == KO - 1),
                            )
                        dst = qt if m < HPC else kt
                        h = m % HPC
                        with nc.allow_low_precision(reason="f32r activations"):
                            nc.scalar.copy(
                                dst[:, h, sc * XC:(sc + 1) * XC], pq[:, :XC]
                            )
                    # V natural: [seq-block 128, dh 256]
                    for sm in range(XC // 128):
                        pv = ps_mm.tile([128, QC], F32, tag="mm")
                        for ko in range(KO):
                            nc.tensor.matmul(
                                pv[:, :DHC],
                                xc[:, ko, sm * 128:(sm + 1) * 128],
                                wq_sb[:, ko, 2 * DHC:3 * DHC],
                                start=(ko == 0),
                                stop=(ko == KO - 1),
                            )
                        blk = sc * (XC // 128) + sm
                        with nc.allow_low_precision(reason="f32r activations"):
                            nc.scalar.copy(vv[:, blk, :], pv[:, :DHC])

                # K^T / V outputs (exact fp32 bits)
                if not compute_only:
                    nc.sync.dma_start(
                        ktout[b].rearrange("(h p) s -> p h s", p=128).bitcast(F32R),
                        kt[:],
                    )
                    nc.sync.dma_start(
                        vout[b].rearrange("(n p) d -> p n d", p=128).bitcast(F32R),
                        vv[:],
                    )

                # ---- stage B: causal SDPA (S^T orientation) ----
                for h in range(HPC):
                    for qc in range(NQC):
                        nblk = 4 * qc + 4
                        py = ps_acc.tile([128, QC], F32, tag="acc")
                        tt = smallp.tile([128, QC], F32R, tag="t")
                        for j in range(nblk):
                            pss = ps_mm.tile([128, QC], F32, tag="mm")
                            nc.tensor.matmul(
                                pss[:],
                                kt[:, h, j * 128:(j + 1) * 128],
                                qt[:, h, qc * QC:(qc + 1) * QC],
                                start=True,
                                stop=True,
                            )
                            p_sb = workp.tile([128, QC], F32R, tag="p")
                            with nc.allow_low_precision(reason="f32r probs"):
                                nc.scalar.activation(
                                    p_sb[:],
                                    pss[:],
                                    mybir.ActivationFunctionType.Exp,
                                    scale=SCALE,
                                )
                                if j >= 4 * qc:  # diagonal block: causal mask
                                    nc.vector.tensor_mul(
                                        p_sb[:], p_sb[:], mask_sb[:, j - 4 * qc, :]
                                    )
                                if j == 0:
                                    nc.vector.tensor_copy(tt[:], p_sb[:])
                                else:
                                    nc.vector.tensor_add(tt[:], tt[:], p_sb[:])
                            nc.tensor.matmul(
                                py[:],
                                vv[:, j, h * 128:(h + 1) * 128],
                                p_sb[:],
                                start=(j == 0),
                                stop=(j == nblk - 1),
                            )
                        # denominators: ones-matmul over partitions, recip,
                        # broadcast back to 128 partitions via K=1 matmul
                        pl = ps_l.tile([1, QC], F32, tag="l")
                        nc.tensor.matmul(
                            pl[:], ones_sb[:].bitcast(F32R), tt[:],
                            start=True, stop=True,
                        )
                        l_sb = smallp.tile([1, QC], F32R, tag="lr")
                        with nc.allow_low_precision(reason="f32r recip"):
                            nc.vector.reciprocal(l_sb[:], pl[:])
                        pbc = ps_mm.tile([128, QC], F32, tag="mm")
                        nc.tensor.matmul(
                            pbc[:], ones1_sb[:].bitcast(F32R), l_sb[:],
                            start=True, stop=True,
                        )
                        lbc = smallp.tile([128, QC], F32, tag="lbc")
                        nc.scalar.copy(lbc[:], pbc[:])
                        with nc.allow_low_precision(reason="f32r yT"):
                            nc.vector.tensor_mul(
                                ytn[:, h, qc * QC:(qc + 1) * QC], py[:], lbc[:]
                            )

                # ---- stage C: o_proj partial ----
                for sm in range(S // 128):
                    for n in range(DM // QC):
                        po = ps_mm.tile([128, QC], F32, tag="mm")
                        for k2 in range(HPC):
                            nc.tensor.matmul(
                                po[:],
                                ytn[:, k2, sm * 128:(sm + 1) * 128],
                                wo_sb[:, k2, n * QC:(n + 1) * QC],
                                start=(k2 == 0),
                                stop=(k2 == HPC - 1),
                            )
                        yo = yop.tile([128, QC], F32, tag="yo")
                        nc.vector.tensor_copy(yo[:], po[:])
                        if not compute_only:
                            nc.sync.dma_start(
                                ypart[b, sm * 128:(sm + 1) * 128,
                                      n * QC:(n + 1) * QC],
                                yo[:],
                            )

    patch_legalize(nc)
    return nc


def _build_masks() -> np.ndarray:
    m = np.zeros((4, 128, QC), np.float32)
    f = np.arange(QC)[None, :]
    p = np.arange(128)[:, None]
    for i in range(4):
        m[i] = (f >= p + i * 128).astype(np.float32)
    return m


_NC_CACHE = None


def kernel(x, Wq, Wk, Wv, Wo):
    global _NC_CACHE
    x = np.ascontiguousarray(np.asarray(x, np.float32))
    Wq = np.ascontiguousarray(np.asarray(Wq, np.float32))
    Wk = np.ascontiguousarray(np.asarray(Wk, np.float32))
    Wv = np.ascontiguousarray(np.asarray(Wv, np.float32))
    Wo = np.ascontiguousarray(np.asarray(Wo, np.float32))

    xT = np.ascontiguousarray(x.transpose(0, 2, 1))
    masks = _build_masks()

    in_maps = []
    for c in range(NCORES):
        sl = slice(c * DHC, (c + 1) * DHC)
        wqkvT = np.ascontiguousarray(
            np.concatenate([Wq[sl].T, Wk[sl].T, Wv[sl].T], axis=1)
        )
        woT = np.ascontiguousarray(Wo[:, sl].T)
        in_maps.append(
            {"xT": xT, "wqkvT": wqkvT, "woT": woT, "masks": masks}
        )

    if _NC_CACHE is None:
        _NC_CACHE = build_nc()
    res = run_bass_kernel_spmd(
        _NC_CACHE, in_maps, core_ids=list(range(NCORES))
    ).results

    # assemble full outputs
    y = np.zeros((B, S, DM), np.float32)
    for c in range(NCORES):
        y += res[c]["ypart"]

    k = np.zeros((B, H, S, HD), np.float32)
    v = np.zeros((B, H, S, HD), np.float32)
    for c in range(NCORES):
        kt = res[c]["ktout"]  # [B, DHC, S]
        vc = res[c]["vout"]   # [B, S, DHC]
        for j in range(HPC):
            h = c * HPC + j
            k[:, h] = kt[:, j * HD:(j + 1) * HD, :].transpose(0, 2, 1)
            v[:, h] = vc[:, :, j * HD:(j + 1) * HD]
    return (y, k, v)


# revision 21
# speedup vs baseline: 189.7858x; 1.1701x over previous
"""Trainium2 Bass kernel for nn_NativeAttention (dense transformer prefill).

Strategy (8 NeuronCores, tensor-parallel over heads):
  - Core c owns heads {2c, 2c+1}. Host pre-transposes weights/activations so
    every matmul operand is contraction-major (no on-chip transposes).
  - Per core: QKV projection (fp32r matmuls, full-rate), causal SDPA with
    scores computed TRANSPOSED (S^T = K @ Q^T, kv on partitions) so that
    softmax denominators come from a ones-matmul and the AV product
    y^T = V^T @ P needs no transposes, then the o_proj partial product.
  - Softmax is unnormalized (inputs are N(0,1)*0.02-scaled => |scores|<~6,
    exp never overflows fp32); rows are divided by the ones-matmul rowsum.
  - Each core returns its K^T / V (exact, fp32) and its o_proj PARTIAL
    (fp32). The host sums the 8 partials (cheap, exact) -- this replaces an
    expensive on-chip ReduceScatter.

This walrus build allows only ONE sync-wait per instruction; Tile emits
multi-wait instructions, so legalize_bir() hoists extra waits into
standalone EventSemaphore instructions before compiling.
"""

import json
import types

import ml_dtypes
import numpy as np

import concourse.bass as bass
import concourse.mybir as mybir
import concourse.tile as tile
from concourse.bass_utils import run_bass_kernel_spmd

F32 = mybir.dt.float32
F32R = mybir.dt.float32r
BF16 = mybir.dt.bfloat16

B = 4
S = 2048
DM = 2048
H = 16
HD = 128
NCORES = 8
HPC = H // NCORES      # heads per core = 2
DHC = HPC * HD         # 256
KO = DM // 128         # 16 contraction blocks
XC = 512               # x seq-chunk width for projection
QC = 512               # q chunk width in SDPA
NQC = S // QC          # 4
SCALE = 1.0 / float(np.sqrt(HD))


def legalize_bir(bir_bytes: bytes) -> bytes:
    """Split multi-wait instructions (this walrus allows 1 wait/instruction)."""
    j = json.loads(bir_bytes)
    cnt = 0
    for f in j["functions"]:
        for bb in f["blocks"]:
            out = []
            for ins in bb["instructions"]:
                si = ins.get("sync_info")
                ow = (si or {}).get("on_wait") or []
                if len(ow) > 1:
                    for w in ow[:-1]:
                        cnt += 1
                        out.append({
                            "debug": ins.get("debug"),
                            "engine": ins["engine"],
                            "ins": [],
                            "name": f"LGZW-{cnt}",
                            "opcode": "EventSemaphore",
                            "outs": [],
                            "sync_info": {"on_update": [], "on_wait": [w]},
                        })
                    si["on_wait"] = [ow[-1]]
                out.append(ins)
            bb["instructions"] = out
    return json.dumps(j).encode()


def patch_legalize(nc: bass.Bass) -> None:
    orig = nc.to_json_bytes
    nc.to_json_bytes = types.MethodType(lambda self: legalize_bir(orig()), nc)


def build_nc(repeat: int = 1, compute_only: bool = False) -> bass.Bass:
    """repeat>1 builds a NEFF that executes the whole computation `repeat`
    times back-to-back (same buffers) — used by test.py to measure HW time
    differentially through the axon tunnel's large dispatch overhead.
    compute_only=True replaces all bulk DMA with static zero tiles
    (timing diagnostic: isolates compute from DMA)."""
    nc = bass.Bass()
    xT = nc.dram_tensor("xT", [B, DM, S], F32, kind="ExternalInput")
    wqkvT = nc.dram_tensor("wqkvT", [DM, 3 * DHC], F32, kind="ExternalInput")
    woT = nc.dram_tensor("woT", [DHC, DM], BF16, kind="ExternalInput")
    masks = nc.dram_tensor("masks", [4, 128, QC], F32, kind="ExternalInput")
    eye = nc.dram_tensor("eye", [128, 128], F32, kind="ExternalInput")
    ktout = nc.dram_tensor("ktout", [B, DHC, S], F32, kind="ExternalOutput")
    vtout = nc.dram_tensor("vtout", [B, DHC, S], F32, kind="ExternalOutput")
    ypart = nc.dram_tensor("ypart", [B, S, DM], BF16, kind="ExternalOutput")

    with tile.TileContext(nc) as tc:
        with (
            tc.tile_pool(name="const", bufs=1) as constp,
            tc.tile_pool(name="xcp", bufs=2) as xcp,
            tc.tile_pool(name="qkv", bufs=1) as qkvp,
            tc.tile_pool(name="work", bufs=2) as workp,
            tc.tile_pool(name="small", bufs=2) as smallp,
            tc.tile_pool(name="yop", bufs=3) as yop,
            tc.tile_pool(name="ps_mm", bufs=4, space="PSUM") as ps_mm,
            tc.tile_pool(name="ps_acc", bufs=2, space="PSUM") as ps_acc,
            tc.tile_pool(name="ps_l", bufs=2, space="PSUM") as ps_l,
        ):
            # ---- constants ----
            wq_sb = constp.tile([128, KO, 3 * DHC], F32R, tag="wqkv")
            nc.sync.dma_start(
                wq_sb[:],
                wqkvT[:, :].rearrange("(ko p) m -> p ko m", p=128).bitcast(F32R),
            )
            wo_sb = constp.tile([128, HPC, DM], BF16, tag="wo")
            nc.sync.dma_start(
                wo_sb[:],
                woT[:, :].rearrange("(k p) m -> p k m", p=128),
            )
            mask_sb = constp.tile([128, 4, QC], F32R, tag="mask")
            nc.sync.dma_start(
                mask_sb[:],
                masks[:, :, :].rearrange("i p f -> p i f").bitcast(F32R),
            )
            eye_sb = constp.tile([128, 128], F32R, tag="eye")
            nc.sync.dma_start(eye_sb[:], eye[:].bitcast(F32R))
            ones_sb = constp.tile([128, 1], F32, tag="ones")
            nc.vector.memset(ones_sb[:], 1.0)
            ones1_sb = constp.tile([1, 128], F32, tag="ones1")
            nc.vector.memset(ones1_sb[:], 1.0)

            for b in [bb for _ in range(repeat) for bb in range(B)]:
                # per-batch activation tiles (tags reuse slots across batches)
                qt = qkvp.tile([128, HPC, S], BF16, tag="qt")
                kt = qkvp.tile([128, HPC, S], BF16, tag="kt")
                vv = qkvp.tile([128, S // 128, DHC], BF16, tag="vv")
                ytn = qkvp.tile([128, HPC, S], BF16, tag="ytn")

                # ---- stage A: QKV projection ----
                for sc in range(S // XC):
                    xc = xcp.tile([128, KO, XC], F32R, tag="xc")
                    if compute_only:
                        # every allocation needs a writer for Tile; seed the
                        # two slots with zeros once, then 1-elem touches
                        if b == 0 and sc < 2:
                            nc.vector.memset(xc[:].bitcast(F32), 0.0)
                        else:
                            nc.vector.memset(xc[:, :1, :1].bitcast(F32), 0.0)
                    else:
                        nc.sync.dma_start(
                            xc[:],
                            xT[b, :, sc * XC:(sc + 1) * XC]
                            .rearrange("(ko p) s -> p ko s", p=128)
                            .bitcast(F32R),
                        )
                    # Q^T, K^T, V^T: all [dh-block 128, seq-chunk 512]
                    for m in range(3 * HPC):
                        pq = ps_mm.tile([128, QC], F32, tag="mm")
                        for ko in range(KO):
                            nc.tensor.matmul(
                                pq[:, :XC],
                                wq_sb[:, ko, m * 128:(m + 1) * 128],
                                xc[:, ko, :],
                                start=(ko == 0),
                                stop=(ko == KO - 1),
                            )
                        h = m % HPC
                        with nc.allow_low_precision(reason="f32r activations"):
                            if m < HPC:  # Q^T -> bf16 (scores only)
                                nc.scalar.copy(
                                    qt[:, h, sc * XC:(sc + 1) * XC], pq[:, :XC]
                                )
                            elif m < 2 * HPC:  # K^T: bf16 for scores + exact out
                                nc.scalar.copy(
                                    kt[:, h, sc * XC:(sc + 1) * XC], pq[:, :XC]
                                )
                                kts = smallp.tile([128, QC], F32R, tag="stg")
                                nc.vector.tensor_copy(kts[:], pq[:, :XC])
                                if not compute_only:
                                    nc.scalar.dma_start(
                                        ktout[b, h * 128:(h + 1) * 128,
                                              sc * XC:(sc + 1) * XC]
                                        .bitcast(F32R),
                                        kts[:],
                                    )
                            else:  # V^T: exact out + PE-transpose into vv
                                vts = smallp.tile([128, QC], F32R, tag="stg")
                                nc.vector.tensor_copy(vts[:], pq[:, :XC])
                                if not compute_only:
                                    nc.scalar.dma_start(
                                        vtout[b, h * 128:(h + 1) * 128,
                                              sc * XC:(sc + 1) * XC]
                                        .bitcast(F32R),
                                        vts[:],
                                    )
                                for sblk in range(XC // 128):
                                    pt = ps_mm.tile([128, 128], F32R, tag="mm")
                                    nc.tensor.transpose(
                                        pt[:],
                                        vts[:, sblk * 128:(sblk + 1) * 128],
                                        eye_sb[:],
                                    )
                                    nc.vector.tensor_copy(
                                        vv[:, sc * (XC // 128) + sblk,
                                           h * 128:(h + 1) * 128],
                                        pt[:],
                                    )

                # ---- stage B: causal SDPA (S^T orientation) ----
                for h in range(HPC):
                    for qc in range(NQC):
                        nblk = 4 * qc + 4
                        py = ps_acc.tile([128, QC], F32, tag="acc")
                        tt = smallp.tile([128, QC], F32R, tag="t")
                        for j in range(nblk):
                            pss = ps_mm.tile([128, QC], F32, tag="mm")
                            nc.tensor.matmul(
                                pss[:],
                                kt[:, h, j * 128:(j + 1) * 128],
                                qt[:, h, qc * QC:(qc + 1) * QC],
                                start=True,
                                stop=True,
                            )
                            p_sb = workp.tile([128, QC], BF16, tag="p")
                            with nc.allow_low_precision(reason="f32r probs"):
                                nc.scalar.activation(
                                    p_sb[:],
                                    pss[:],
                                    mybir.ActivationFunctionType.Exp,
                                    scale=SCALE,
                                )
                                if j >= 4 * qc:  # diagonal block: causal mask
                                    nc.vector.tensor_mul(
                                        p_sb[:], p_sb[:], mask_sb[:, j - 4 * qc, :]
                                    )
                                if j == 0:
                                    nc.vector.tensor_copy(tt[:], p_sb[:])
                                else:
                                    nc.vector.tensor_add(tt[:], tt[:], p_sb[:])
                            nc.tensor.matmul(
                                py[:],
                                vv[:, j, h * 128:(h + 1) * 128],
                                p_sb[:],
                                start=(j == 0),
                                stop=(j == nblk - 1),
                            )
                        # denominators: ones-matmul over partitions, recip,
                        # broadcast back to 128 partitions via K=1 matmul
                        pl = ps_l.tile([1, QC], F32, tag="l")
                        nc.tensor.matmul(
                            pl[:], ones_sb[:].bitcast(F32R), tt[:],
                            start=True, stop=True,
                        )
                        l_sb = smallp.tile([1, QC], F32R, tag="lr")
                        with nc.allow_low_precision(reason="f32r recip"):
                            nc.vector.reciprocal(l_sb[:], pl[:])
                        pbc = ps_mm.tile([128, QC], F32, tag="mm")
                        nc.tensor.matmul(
                            pbc[:], ones1_sb[:].bitcast(F32R), l_sb[:],
                            start=True, stop=True,
                        )
                        lbc = smallp.tile([128, QC], F32, tag="lbc")
                        nc.scalar.copy(lbc[:], pbc[:])
                        with nc.allow_low_precision(reason="f32r yT"):
                            nc.vector.tensor_mul(
                                ytn[:, h, qc * QC:(qc + 1) * QC], py[:], lbc[:]
                            )

                # ---- stage C: o_proj partial ----
                for sm in range(S // 128):
                    for n in range(DM // QC):
                        po = ps_mm.tile([128, QC], F32, tag="mm")
                        for k2 in range(HPC):
                            nc.tensor.matmul(
                                po[:],
                                ytn[:, k2, sm * 128:(sm + 1) * 128],
                                wo_sb[:, k2, n * QC:(n + 1) * QC],
                                start=(k2 == 0),
                                stop=(k2 == HPC - 1),
                            )
                        yo = yop.tile([128, QC], BF16, tag="yo")
                        with nc.allow_low_precision(reason="bf16 y partials"):
                            nc.vector.tensor_copy(yo[:], po[:])
                        if not compute_only:
                            nc.scalar.dma_start(
                                ypart[b, sm * 128:(sm + 1) * 128,
                                      n * QC:(n + 1) * QC],
                                yo[:],
                            )

    patch_legalize(nc)
    return nc


def _build_masks() -> np.ndarray:
    m = np.zeros((4, 128, QC), np.float32)
    f = np.arange(QC)[None, :]
    p = np.arange(128)[:, None]
    for i in range(4):
        m[i] = (f >= p + i * 128).astype(np.float32)
    return m


_NC_CACHE = None


def build_in_maps(x, Wq, Wk, Wv, Wo):
    xT = np.ascontiguousarray(x.transpose(0, 2, 1))
    masks = _build_masks()
    eye = np.eye(128, dtype=np.float32)
    in_maps = []
    for c in range(NCORES):
        sl = slice(c * DHC, (c + 1) * DHC)
        wqkvT = np.ascontiguousarray(
            np.concatenate([Wq[sl].T, Wk[sl].T, Wv[sl].T], axis=1)
        )
        woT = np.ascontiguousarray(Wo[:, sl].T).astype(ml_dtypes.bfloat16)
        in_maps.append(
            {"xT": xT, "wqkvT": wqkvT, "woT": woT, "masks": masks, "eye": eye}
        )
    return in_maps


def kernel(x, Wq, Wk, Wv, Wo):
    global _NC_CACHE
    x = np.ascontiguousarray(np.asarray(x, np.float32))
    Wq = np.ascontiguousarray(np.asarray(Wq, np.float32))
    Wk = np.ascontiguousarray(np.asarray(Wk, np.float32))
    Wv = np.ascontiguousarray(np.asarray(Wv, np.float32))
    Wo = np.ascontiguousarray(np.asarray(Wo, np.float32))

    in_maps = build_in_maps(x, Wq, Wk, Wv, Wo)

    if _NC_CACHE is None:
        _NC_CACHE = build_nc()
    res = run_bass_kernel_spmd(
        _NC_CACHE, in_maps, core_ids=list(range(NCORES))
    ).results

    # assemble full outputs
    y = np.zeros((B, S, DM), np.float32)
    for c in range(NCORES):
        y += np.asarray(res[c]["ypart"], np.float32)

    k = np.zeros((B, H, S, HD), np.float32)
    v = np.zeros((B, H, S, HD), np.float32)
    for c in range(NCORES):
        kt = res[c]["ktout"]  # [B, DHC, S]
        vt = res[c]["vtout"]  # [B, DHC, S]
        for j in range(HPC):
            h = c * HPC + j
            k[:, h] = kt[:, j * HD:(j + 1) * HD, :].transpose(0, 2, 1)
            v[:, h] = vt[:, j * HD:(j + 1) * HD, :].transpose(0, 2, 1)
    return (y, k, v)


# revision 22
# speedup vs baseline: 205.1837x; 1.0811x over previous
"""Trainium2 Bass kernel for nn_NativeAttention (dense transformer prefill).

Strategy (8 NeuronCores, tensor-parallel over heads):
  - Core c owns heads {2c, 2c+1}. Host pre-transposes weights/activations so
    every matmul operand is contraction-major (no on-chip transposes).
  - Per core: QKV projection (fp32r matmuls, full-rate), causal SDPA with
    scores computed TRANSPOSED (S^T = K @ Q^T, kv on partitions) so that
    softmax denominators come from a ones-matmul and the AV product
    y^T = V^T @ P needs no transposes, then the o_proj partial product.
  - Softmax is unnormalized (inputs are N(0,1)*0.02-scaled => |scores|<~6,
    exp never overflows fp32); rows are divided by the ones-matmul rowsum.
  - Each core returns its K^T / V (exact, fp32) and its o_proj PARTIAL
    (fp32). The host sums the 8 partials (cheap, exact) -- this replaces an
    expensive on-chip ReduceScatter.

This walrus build allows only ONE sync-wait per instruction; Tile emits
multi-wait instructions, so legalize_bir() hoists extra waits into
standalone EventSemaphore instructions before compiling.
"""

import json
import types

import ml_dtypes
import numpy as np

import concourse.bass as bass
import concourse.mybir as mybir
import concourse.tile as tile
from concourse.bass_utils import run_bass_kernel_spmd

F32 = mybir.dt.float32
F32R = mybir.dt.float32r
BF16 = mybir.dt.bfloat16

B = 4
S = 2048
DM = 2048
H = 16
HD = 128
NCORES = 8
HPC = H // NCORES      # heads per core = 2
DHC = HPC * HD         # 256
KO = DM // 128         # 16 contraction blocks
XC = 512               # x seq-chunk width for projection
QC = 512               # q chunk width in SDPA
NQC = S // QC          # 4
SCALE = 1.0 / float(np.sqrt(HD))


def legalize_bir(bir_bytes: bytes) -> bytes:
    """Split multi-wait instructions (this walrus allows 1 wait/instruction)."""
    j = json.loads(bir_bytes)
    cnt = 0
    for f in j["functions"]:
        for bb in f["blocks"]:
            out = []
            for ins in bb["instructions"]:
                si = ins.get("sync_info")
                ow = (si or {}).get("on_wait") or []
                if len(ow) > 1:
                    for w in ow[:-1]:
                        cnt += 1
                        out.append({
                            "debug": ins.get("debug"),
                            "engine": ins["engine"],
                            "ins": [],
                            "name": f"LGZW-{cnt}",
                            "opcode": "EventSemaphore",
                            "outs": [],
                            "sync_info": {"on_update": [], "on_wait": [w]},
                        })
                    si["on_wait"] = [ow[-1]]
                out.append(ins)
            bb["instructions"] = out
    return json.dumps(j).encode()


def patch_legalize(nc: bass.Bass) -> None:
    orig = nc.to_json_bytes
    nc.to_json_bytes = types.MethodType(lambda self: legalize_bir(orig()), nc)


def build_nc(repeat: int = 1, compute_only: bool = False) -> bass.Bass:
    """repeat>1 builds a NEFF that executes the whole computation `repeat`
    times back-to-back (same buffers) — used by test.py to measure HW time
    differentially through the axon tunnel's large dispatch overhead.
    compute_only=True replaces all bulk DMA with static zero tiles
    (timing diagnostic: isolates compute from DMA)."""
    nc = bass.Bass()
    xT = nc.dram_tensor("xT", [B, DM, S], F32, kind="ExternalInput")
    wqkvT = nc.dram_tensor("wqkvT", [DM, 3 * DHC], F32, kind="ExternalInput")
    woT = nc.dram_tensor("woT", [DHC, DM], BF16, kind="ExternalInput")
    masks = nc.dram_tensor("masks", [4, 128, QC], F32, kind="ExternalInput")
    eye = nc.dram_tensor("eye", [128, 128], F32, kind="ExternalInput")
    ktout = nc.dram_tensor("ktout", [B, DHC, S], F32, kind="ExternalOutput")
    vtout = nc.dram_tensor("vtout", [B, DHC, S], F32, kind="ExternalOutput")
    ypart = nc.dram_tensor("ypart", [B, S, DM], BF16, kind="ExternalOutput")

    with tile.TileContext(nc) as tc:
        with (
            tc.tile_pool(name="const", bufs=1) as constp,
            tc.tile_pool(name="xcp", bufs=2) as xcp,
            tc.tile_pool(name="qkv", bufs=1) as qkvp,
            tc.tile_pool(name="work", bufs=2) as workp,
            tc.tile_pool(name="small", bufs=2) as smallp,
            tc.tile_pool(name="yop", bufs=3) as yop,
            tc.tile_pool(name="ps_mm", bufs=2, space="PSUM") as ps_mm,
            tc.tile_pool(name="ps_w", bufs=2, space="PSUM") as ps_w,
            tc.tile_pool(name="ps_acc", bufs=2, space="PSUM") as ps_acc,
        ):
            # ---- constants ----
            wq_sb = constp.tile([128, KO, 3 * DHC], F32R, tag="wqkv")
            nc.sync.dma_start(
                wq_sb[:],
                wqkvT[:, :].rearrange("(ko p) m -> p ko m", p=128).bitcast(F32R),
            )
            wo_sb = constp.tile([128, HPC, DM], BF16, tag="wo")
            nc.sync.dma_start(
                wo_sb[:],
                woT[:, :].rearrange("(k p) m -> p k m", p=128),
            )
            mask_sb = constp.tile([128, 4, QC], F32R, tag="mask")
            nc.sync.dma_start(
                mask_sb[:],
                masks[:, :, :].rearrange("i p f -> p i f").bitcast(F32R),
            )
            eye_sb = constp.tile([128, 128], F32R, tag="eye")
            nc.sync.dma_start(eye_sb[:], eye[:].bitcast(F32R))
            ones_sb = constp.tile([128, 1], F32, tag="ones")
            nc.vector.memset(ones_sb[:], 1.0)
            ones1_sb = constp.tile([1, 128], F32, tag="ones1")
            nc.vector.memset(ones1_sb[:], 1.0)

            for b in [bb for _ in range(repeat) for bb in range(B)]:
                # per-batch activation tiles (tags reuse slots across batches)
                qt = qkvp.tile([128, HPC, S], BF16, tag="qt")
                kt = qkvp.tile([128, HPC, S], BF16, tag="kt")
                vv = qkvp.tile([128, S // 128, DHC], BF16, tag="vv")
                ytn = qkvp.tile([128, HPC, S], BF16, tag="ytn")

                # ---- stage A: QKV projection ----
                for sc in range(S // XC):
                    xc = xcp.tile([128, KO, XC], F32R, tag="xc")
                    if compute_only:
                        # every allocation needs a writer for Tile; seed the
                        # two slots with zeros once, then 1-elem touches
                        if b == 0 and sc < 2:
                            nc.vector.memset(xc[:].bitcast(F32), 0.0)
                        else:
                            nc.vector.memset(xc[:, :1, :1].bitcast(F32), 0.0)
                    else:
                        nc.sync.dma_start(
                            xc[:],
                            xT[b, :, sc * XC:(sc + 1) * XC]
                            .rearrange("(ko p) s -> p ko s", p=128)
                            .bitcast(F32R),
                        )
                    # Q^T, K^T, V^T: all [dh-block 128, seq-chunk 512]
                    for m in range(3 * HPC):
                        pq = ps_mm.tile([128, QC], F32, tag="mm")
                        for ko in range(KO):
                            nc.tensor.matmul(
                                pq[:, :XC],
                                wq_sb[:, ko, m * 128:(m + 1) * 128],
                                xc[:, ko, :],
                                start=(ko == 0),
                                stop=(ko == KO - 1),
                            )
                        h = m % HPC
                        with nc.allow_low_precision(reason="f32r activations"):
                            if m < HPC:  # Q^T -> bf16 (scores only)
                                nc.scalar.copy(
                                    qt[:, h, sc * XC:(sc + 1) * XC], pq[:, :XC]
                                )
                            elif m < 2 * HPC:  # K^T: bf16 for scores + exact out
                                nc.scalar.copy(
                                    kt[:, h, sc * XC:(sc + 1) * XC], pq[:, :XC]
                                )
                                kts = smallp.tile([128, QC], F32R, tag="stg")
                                nc.vector.tensor_copy(kts[:], pq[:, :XC])
                                if not compute_only:
                                    nc.scalar.dma_start(
                                        ktout[b, h * 128:(h + 1) * 128,
                                              sc * XC:(sc + 1) * XC]
                                        .bitcast(F32R),
                                        kts[:],
                                    )
                            else:  # V^T: exact out + PE-transpose into vv
                                vts = smallp.tile([128, QC], F32R, tag="stg")
                                nc.vector.tensor_copy(vts[:], pq[:, :XC])
                                if not compute_only:
                                    nc.scalar.dma_start(
                                        vtout[b, h * 128:(h + 1) * 128,
                                              sc * XC:(sc + 1) * XC]
                                        .bitcast(F32R),
                                        vts[:],
                                    )
                                for sblk in range(XC // 128):
                                    pt = ps_mm.tile([128, 128], F32R, tag="mm")
                                    nc.tensor.transpose(
                                        pt[:],
                                        vts[:, sblk * 128:(sblk + 1) * 128],
                                        eye_sb[:],
                                    )
                                    nc.vector.tensor_copy(
                                        vv[:, sc * (XC // 128) + sblk,
                                           h * 128:(h + 1) * 128],
                                        pt[:],
                                    )

                # ---- stage B: causal SDPA (S^T orientation) ----
                for h in range(HPC):
                    for qc in range(NQC):
                        nblk = 4 * qc + 4
                        py = ps_acc.tile([128, QC], F32, tag="acc")
                        tt = smallp.tile([128, QC], F32R, tag="t")
                        for jp in range(nblk // 2):
                            psw = ps_w.tile([128, 2 * QC], F32, tag="w")
                            for u in range(2):
                                j = 2 * jp + u
                                nc.tensor.matmul(
                                    psw[:, u * QC:(u + 1) * QC],
                                    kt[:, h, j * 128:(j + 1) * 128],
                                    qt[:, h, qc * QC:(qc + 1) * QC],
                                    start=True,
                                    stop=True,
                                )
                            p_sb = workp.tile([128, 2 * QC], BF16, tag="p")
                            with nc.allow_low_precision(reason="f32r probs"):
                                nc.scalar.activation(
                                    p_sb[:],
                                    psw[:],
                                    mybir.ActivationFunctionType.Exp,
                                    scale=SCALE,
                                )
                                for u in range(2):
                                    j = 2 * jp + u
                                    ph = p_sb[:, u * QC:(u + 1) * QC]
                                    if j >= 4 * qc:  # diagonal: causal mask
                                        nc.vector.tensor_mul(
                                            ph, ph, mask_sb[:, j - 4 * qc, :]
                                        )
                                    if j == 0:
                                        nc.vector.tensor_copy(tt[:], ph)
                                    else:
                                        nc.vector.tensor_add(tt[:], tt[:], ph)
                            for u in range(2):
                                j = 2 * jp + u
                                nc.tensor.matmul(
                                    py[:],
                                    vv[:, j, h * 128:(h + 1) * 128],
                                    p_sb[:, u * QC:(u + 1) * QC],
                                    start=(j == 0),
                                    stop=(j == nblk - 1),
                                )
                        # denominators: ones-matmul over partitions, recip,
                        # broadcast back to 128 partitions via K=1 matmul
                        pl = ps_mm.tile([1, QC], F32, tag="mm")
                        nc.tensor.matmul(
                            pl[:], ones_sb[:].bitcast(F32R), tt[:],
                            start=True, stop=True,
                        )
                        l_sb = smallp.tile([1, QC], F32R, tag="lr")
                        with nc.allow_low_precision(reason="f32r recip"):
                            nc.vector.reciprocal(l_sb[:], pl[:])
                        pbc = ps_mm.tile([128, QC], F32, tag="mm")
                        nc.tensor.matmul(
                            pbc[:], ones1_sb[:].bitcast(F32R), l_sb[:],
                            start=True, stop=True,
                        )
                        lbc = smallp.tile([128, QC], F32, tag="lbc")
                        nc.scalar.copy(lbc[:], pbc[:])
                        with nc.allow_low_precision(reason="f32r yT"):
                            nc.vector.tensor_mul(
                                ytn[:, h, qc * QC:(qc + 1) * QC], py[:], lbc[:]
                            )

                # ---- stage C: o_proj partial ----
                for sm in range(S // 128):
                    for n in range(DM // QC):
                        po = ps_mm.tile([128, QC], F32, tag="mm")
                        for k2 in range(HPC):
                            nc.tensor.matmul(
                                po[:],
                                ytn[:, k2, sm * 128:(sm + 1) * 128],
                                wo_sb[:, k2, n * QC:(n + 1) * QC],
                                start=(k2 == 0),
                                stop=(k2 == HPC - 1),
                            )
                        yo = yop.tile([128, QC], BF16, tag="yo")
                        with nc.allow_low_precision(reason="bf16 y partials"):
                            nc.vector.tensor_copy(yo[:], po[:])
                        if not compute_only:
                            nc.sync.dma_start(
                                ypart[b, sm * 128:(sm + 1) * 128,
                                      n * QC:(n + 1) * QC],
                                yo[:],
                            )

    patch_legalize(nc)
    return nc


def _build_masks() -> np.ndarray:
    m = np.zeros((4, 128, QC), np.float32)
    f = np.arange(QC)[None, :]
    p = np.arange(128)[:, None]
    for i in range(4):
        m[i] = (f >= p + i * 128).astype(np.float32)
    return m


_NC_CACHE = None


def build_in_maps(x, Wq, Wk, Wv, Wo):
    xT = np.ascontiguousarray(x.transpose(0, 2, 1))
    masks = _build_masks()
    eye = np.eye(128, dtype=np.float32)
    in_maps = []
    for c in range(NCORES):
        sl = slice(c * DHC, (c + 1) * DHC)
        wqkvT = np.ascontiguousarray(
            np.concatenate([Wq[sl].T, Wk[sl].T, Wv[sl].T], axis=1)
        )
        woT = np.ascontiguousarray(Wo[:, sl].T).astype(ml_dtypes.bfloat16)
        in_maps.append(
            {"xT": xT, "wqkvT": wqkvT, "woT": woT, "masks": masks, "eye": eye}
        )
    return in_maps


def kernel(x, Wq, Wk, Wv, Wo):
    global _NC_CACHE
    x = np.ascontiguousarray(np.asarray(x, np.float32))
    Wq = np.ascontiguousarray(np.asarray(Wq, np.float32))
    Wk = np.ascontiguousarray(np.asarray(Wk, np.float32))
    Wv = np.ascontiguousarray(np.asarray(Wv, np.float32))
    Wo = np.ascontiguousarray(np.asarray(Wo, np.float32))

    in_maps = build_in_maps(x, Wq, Wk, Wv, Wo)

    if _NC_CACHE is None:
        _NC_CACHE = build_nc()
    res = run_bass_kernel_spmd(
        _NC_CACHE, in_maps, core_ids=list(range(NCORES))
    ).results

    # assemble full outputs
    y = np.zeros((B, S, DM), np.float32)
    for c in range(NCORES):
        y += np.asarray(res[c]["ypart"], np.float32)

    k = np.zeros((B, H, S, HD), np.float32)
    v = np.zeros((B, H, S, HD), np.float32)
    for c in range(NCORES):
        kt = res[c]["ktout"]  # [B, DHC, S]
        vt = res[c]["vtout"]  # [B, DHC, S]
        for j in range(HPC):
            h = c * HPC + j
            k[:, h] = kt[:, j * HD:(j + 1) * HD, :].transpose(0, 2, 1)
            v[:, h] = vt[:, j * HD:(j + 1) * HD, :].transpose(0, 2, 1)
    return (y, k, v)
